# revision 40
# baseline (speedup 1.0000x reference)
"""Multi-head attention (B=4, S=2048, E=1024, H=16, D=64) on 8 Trainium2 cores.

Sharding: 8 cores = 4 batches x 2 head-halves (data parallel on B, tensor
parallel on heads: each core handles 8 heads = 512 of the 1024 QKV columns /
out-proj rows for one batch). Each core returns a partial [S, E] output
(its half of the out-projection contraction); the host sums core pairs.

Device algorithm per core (bf16 matmul operands, fp32 accumulation in
PSUM, fp32 output):
  - Q/K/V projected chunk-by-chunk (512 seq positions at a time) into
    head-pair layout: KT/QT [128, 4, S] (partition = d-within-pair), V in
    natural [S, 512] layout stored [128, 16, 8*65] with a ones-column per
    head (softmax denominators fall out as row 64 of the PV psum).
  - Causal interleave: attention on q-tile t only needs K/V chunks <= t,
    so the loop runs  scores(t) ; project chunk t+1 ; normalize(t)  and
    the Tile scheduler fills attention's exp-wait PE bubbles with
    projection matmuls, keeping TensorE dense (HAM stays un-throttled).
    Out-projections of q-tiles 0..2 are deferred into the last q-tile's
    scores phase (after pairs 1..3), which has no projection filler left.
  - Scores transposed: ST[k, q] = KT-block^T . QT per (head, k-block 128,
    q-tile 512), the two heads of a pair at tile_position (0,0)/(64,0).
    exp on ScalarE (scale=1/8) over [128, <=1024] kb-pair tiles -> bf16.
  - Causal mask: ST/exp/PV are width-trimmed to the valid suffix per
    diagonal block; the boundary 128x128 block is zeroed above the
    diagonal by one bf16 triangular multiply. No memsets, no -inf.
  - PV: OT_aug[65, q] += V_aug^T . PT accumulated over k in PSUM; row 64 =
    denominator. The 8 denominators of a q-tile are packed at 32-aligned
    partitions of two [128, 512] tiles (memset to 1.0 so untouched rows
    stay finite) -> one batched DVE reciprocal per tile (kicked off as
    soon as its pairs finish) -> one K=128 matmul against a selection
    matrix broadcasts 1/den to [128, q] per head pair, reusing the
    evacuated pv banks so the mm PSUM tag stays free for projections.
  - Out-projection: out[q, e] += OT_pair_j^T . wo[128j:+128, e]; output
    DMAs ride the Sync HWDGE ring behind all input loads.

The mask input is honored: causal and all-ones masks run on device;
anything else (or nonzero biases) falls back to an exact numpy
implementation (never hit by the harness).
"""

import os
import sys

sys.path.insert(0, "/opt/trn_rl_repo")

import numpy as np

B, S, E, H = 4, 2048, 1024, 16
D = E // H  # 64
P = 128
KO = E // P          # 8 contraction chunks for projections
NJ = 4               # head pairs per core
SQT = 512            # q tile
NQT = S // SQT       # 4
NKB = S // P         # 16 k blocks
EH = E // 2          # 512 columns per core

_CACHE = {}
LAST_RESULT = None   # BassKernelResults of the most recent device run


def _build(causal: bool):
    import concourse.bass as bass  # noqa: F401
    import concourse.mybir as mybir
    import concourse.tile as tile
    from concourse import bacc
    from contextlib import ExitStack

    f32 = mybir.dt.float32
    rdt = mybir.dt.float32r
    bf16 = mybir.dt.bfloat16
    AF = mybir.ActivationFunctionType

    nc = bacc.Bacc("TRN2", target_bir_lowering=False, debug=False, num_devices=8)

    xt_q = nc.dram_tensor("xt_q", [E, S], bf16, kind="ExternalInput")
    xt_k = nc.dram_tensor("xt_k", [E, S], bf16, kind="ExternalInput")
    xt_v = nc.dram_tensor("xt_v", [E, S], bf16, kind="ExternalInput")
    wq_d = nc.dram_tensor("wq_h", [E, EH], bf16, kind="ExternalInput")
    wk_d = nc.dram_tensor("wk_h", [E, EH], bf16, kind="ExternalInput")
    wv_d = nc.dram_tensor("wv_h", [E, EH], bf16, kind="ExternalInput")
    wo_d = nc.dram_tensor("wo_h", [EH, E], bf16, kind="ExternalInput")
    tri_d = nc.dram_tensor("tri", [P, P], bf16, kind="ExternalInput")
    sel_d = nc.dram_tensor("selmat", [P, NJ * P], f32, kind="ExternalInput")
    out_d = nc.dram_tensor("out", [S, E], f32, kind="ExternalOutput")

    def rcast(ap):
        return ap.bitcast(rdt)

    with nc.allow_low_precision(reason="f32r/bf16 matmul inputs"), \
            tile.TileContext(nc) as tc, ExitStack() as top:
        consts = top.enter_context(tc.tile_pool(name="consts", bufs=1))
        big = top.enter_context(tc.tile_pool(name="big", bufs=1))
        xtp = top.enter_context(tc.tile_pool(name="xtp", bufs=3))
        wp = top.enter_context(tc.tile_pool(name="wp", bufs=1))
        qtp = top.enter_context(tc.tile_pool(name="qtp", bufs=2))
        ptp = top.enter_context(tc.tile_pool(name="ptp", bufs=3))
        pvcp = top.enter_context(tc.tile_pool(name="pvcp", bufs=8))
        denp = top.enter_context(tc.tile_pool(name="denp", bufs=2))
        otp = top.enter_context(tc.tile_pool(name="otp", bufs=16))
        osbp = top.enter_context(tc.tile_pool(name="osbp", bufs=4))
        # PSUM budget (8 banks): st0/st1 bank-pairs = 4, pv0/pv1 = 2,
        # shared "mm" tag (projection groups, rep broadcast, out-proj) = 2.
        st_ps = top.enter_context(tc.tile_pool(name="st_ps", bufs=1, space="PSUM"))
        pv_ps = top.enter_context(tc.tile_pool(name="pv_ps", bufs=1, space="PSUM"))
        mm_ps = top.enter_context(tc.tile_pool(name="mm_ps", bufs=2, space="PSUM"))

        tri_sb = consts.tile([P, P], bf16, tag="tri")
        nc.sync.dma_start(tri_sb[:], tri_d.ap())
        sel_sb = consts.tile([P, NJ * P], rdt, tag="sel")
        nc.sync.dma_start(sel_sb[:], rcast(sel_d.ap()))
        wo_sb = consts.tile([P, NJ, E], bf16, tag="wo")

        kt_sb = big.tile([P, NJ, S], bf16, tag="KT")
        v_sb = big.tile([P, NKB, 8 * (D + 1)], bf16, tag="V")
        # ones column per head at index 64 of each 65-wide head group
        v_view = v_sb[:].rearrange("p b (h w) -> p b h w", h=8)
        nc.gpsimd.memset(v_view[:, :, :, D : D + 1], 1.0)

        def load_w(w_dram, wtag, interleave_with=None):
            """DMA a weight [E, EH] into its tile. If interleave_with=
            (xt_dram, sc), alternate weight and xt chunk DMAs so the first
            matmul group can start after ~2 chunks land."""
            w_t = wp.tile([P, KO, EH], bf16, tag=wtag, name=wtag)
            xt_t = None
            if interleave_with is not None:
                xt_dram, sc = interleave_with
                xt_t = xtp.tile([P, KO, SQT], bf16, tag="xt", name="xt_t")
            for ko in range(KO):
                nc.sync.dma_start(
                    w_t[:, ko, :],
                    w_dram.ap()[ko * P : (ko + 1) * P, :],
                )
                if xt_t is not None:
                    nc.sync.dma_start(
                        xt_t[:, ko, :],
                        xt_dram.ap()[
                            ko * P : (ko + 1) * P, sc * SQT : (sc + 1) * SQT
                        ],
                    )
            return w_t, xt_t

        def load_xt(xt_dram, sc):
            xt_t = xtp.tile([P, KO, SQT], bf16, tag="xt", name="xt_t")
            for ko in range(KO):
                nc.sync.dma_start(
                    xt_t[:, ko, :],
                    xt_dram.ap()[
                        ko * P : (ko + 1) * P, sc * SQT : (sc + 1) * SQT
                    ],
                )
            return xt_t

        def proj_dt(w_t, xt_t, dst, dst_col):
            """One [d-pairs, 512] chunk of QT or KT into dst at dst_col."""
            for j in range(NJ):
                pst = mm_ps.tile([P, SQT], f32, tag="mm", name="pst")
                for ko in range(KO):
                    nc.tensor.matmul(
                        pst[:],
                        w_t[:, ko, j * P : (j + 1) * P],
                        xt_t[:, ko, :],
                        start=(ko == 0),
                        stop=(ko == KO - 1),
                    )
                nc.vector.tensor_copy(
                    dst[:, j, dst_col : dst_col + SQT], pst[:]
                )

        def proj_q(w_t, sc):
            qt_t = qtp.tile([P, NJ, SQT], bf16, tag="qt", name="qt_t")
            proj_dt(w_t, load_xt(xt_q, sc), qt_t, 0)
            return qt_t

        def proj_v_chunk(w_t, xt_t, sc):
            for sb in range(4):
                pst = mm_ps.tile([P, EH], f32, tag="mm", name="pst")
                for ko in range(KO):
                    nc.tensor.matmul(
                        pst[:],
                        xt_t[:, ko, sb * P : (sb + 1) * P],
                        w_t[:, ko, :],
                        start=(ko == 0),
                        stop=(ko == KO - 1),
                    )
                sblk = 4 * sc + sb
                nc.vector.tensor_copy(
                    v_sb[:, sblk, :].rearrange("p (h w) -> p h w", h=8)[:, :, 0:D],
                    pst[:].rearrange("p (h w) -> p h w", h=8),
                )

        def attn_scores_pv(qt, qt_t, fillers=()):
            """ST/exp/PV for one q-tile; evacuates each pair's PV and kicks
            off the batched reciprocals as soon as their denominators are
            ready. fillers[j]() emits independent PE work (deferred
            out-projections) after pair j, giving the scheduler matmuls to
            slot into this tile's exp-wait bubbles. Returns (pvc_all, inv_t).
            """
            nkb = 4 * (qt + 1) if causal else NKB

            def trim(kb):
                if not causal:
                    return 0
                return P * max(0, kb - 4 * qt)

            den_t = []
            for jj in (0, 1):
                dt_ = denp.tile([P, SQT], f32, tag=f"den{jj}", name=f"den{jj}")
                nc.gpsimd.memset(dt_[:], 1.0)
                den_t.append(dt_)
            inv_t = [None, None]
            pvc_all = []
            for j in range(NJ):
                pv = [
                    pv_ps.tile([D + 1, SQT], f32, tag=f"pv{h2}", name=f"pv{h2}")
                    for h2 in (0, 1)
                ]
                for kbp in range(nkb // 2):
                    kbs = (2 * kbp, 2 * kbp + 1)
                    st = [
                        st_ps.tile([P, 2 * SQT], f32, tag=f"st{h2}", name=f"st{h2}")
                        for h2 in (0, 1)
                    ]
                    # interleave the two head-halves so each LDWEIGHTS
                    # overlaps the other half's in-flight matmul
                    for i, kb in enumerate(kbs):
                        c0 = trim(kb)
                        for h2 in (0, 1):
                            nc.tensor.matmul(
                                st[h2][:, i * SQT + c0 : (i + 1) * SQT],
                                kt_sb[
                                    h2 * D : (h2 + 1) * D,
                                    j,
                                    kb * P : (kb + 1) * P,
                                ],
                                qt_t[h2 * D : (h2 + 1) * D, j, c0:SQT],
                                start=True,
                                stop=True,
                                tile_position=(h2 * D, 0),
                            )
                    for h2 in (0, 1):
                        h = 2 * j + h2
                        pt = ptp.tile(
                            [P, 2 * SQT], bf16, tag=f"pt{h2}", name=f"pt{h2}"
                        )
                        a0 = trim(kbs[0])
                        nc.scalar.activation(
                            pt[:, a0 : 2 * SQT],
                            st[h2][:, a0 : 2 * SQT],
                            AF.Exp,
                            scale=0.125,
                        )
                        for i, kb in enumerate(kbs):
                            c0 = trim(kb)
                            p_ = kb - 4 * qt
                            if causal and p_ >= 0:
                                blk = slice(
                                    i * SQT + c0, i * SQT + c0 + P
                                )
                                nc.vector.tensor_mul(
                                    pt[:, blk], pt[:, blk], tri_sb[:]
                                )
                            nc.tensor.matmul(
                                pv[h2][:, c0:SQT],
                                v_sb[:, kb, h * (D + 1) : (h + 1) * (D + 1)],
                                pt[:, i * SQT + c0 : (i + 1) * SQT],
                                start=(kb == 0),
                                stop=(kb == nkb - 1),
                            )
                # evacuate this pair's PV on ScalarE (keeps the DVE free for
                # the reciprocals) and pack the denominators at 32-aligned
                # partitions of the [128, 512] den tiles (memset to 1.0 so
                # untouched rows stay finite).
                pvc = [
                    pvcp.tile([D, SQT], f32, tag="pvc", name=f"pvc{j}{h2}")
                    for h2 in (0, 1)
                ]
                pvc_all.append(pvc)
                # den copies (and the reciprocal they gate) first: the pvc
                # evacuations are off the critical chain to the broadcasts
                for h2 in (0, 1):
                    r = 64 * (j % 2) + 32 * h2
                    nc.vector.tensor_copy(
                        den_t[j // 2][r : r + 1, :],
                        pv[h2][D : D + 1, :],
                    )
                if j % 2 == 1:
                    jj = j // 2
                    iv = denp.tile(
                        [P, SQT], rdt, tag=f"inv{jj}", name=f"inv{jj}"
                    )
                    nc.vector.reciprocal(iv[:], den_t[jj][:])
                    inv_t[jj] = iv
                for h2 in (0, 1):
                    nc.vector.tensor_copy(pvc[h2][:], pv[h2][0:D, :])
                # emit after pairs 1..3 so the last filler lands over the
                # final reciprocal stall (keeps the PE warm into the tail)
                if j >= 1 and j - 1 < len(fillers):
                    fillers[j - 1]()
            return pvc_all, inv_t

        def attn_norm(pvc_all, inv_t):
            # one K=128 selection matmul per pair broadcasts 1/den to [128, q]
            ot_tiles = []
            for j in range(NJ):
                # reuse the pv banks (evacuated by now) so the mm tag stays
                # free for projection/out-proj groups during normalization
                rp = pv_ps.tile([P, SQT], f32, tag=f"pv{j % 2}", name="rp")
                nc.tensor.matmul(
                    rp[:],
                    sel_sb[:, j * P : (j + 1) * P],
                    inv_t[j // 2][:],
                    start=True,
                    stop=True,
                )
                ot = otp.tile([P, SQT], bf16, tag="ot", name="ot")
                ot_tiles.append(ot)
                for h2 in (0, 1):
                    nc.vector.tensor_mul(
                        ot[h2 * D : (h2 + 1) * D, :],
                        pvc_all[j][h2][:],
                        rp[h2 * D : (h2 + 1) * D, :],
                    )
            return ot_tiles

        def outproj_qt(qt, ot_tiles):
            for qb in range(4):
                for ec in range(2):
                    ops = mm_ps.tile([P, SQT], f32, tag="mm", name="ops")
                    for j in range(NJ):
                        nc.tensor.matmul(
                            ops[:],
                            ot_tiles[j][:, qb * P : (qb + 1) * P],
                            wo_sb[:, j, ec * SQT : (ec + 1) * SQT],
                            start=(j == 0),
                            stop=(j == NJ - 1),
                        )
                    osb = osbp.tile([P, SQT], f32, tag="osb", name="osb")
                    nc.vector.tensor_copy(osb[:], ops[:])
                    nc.sync.dma_start(
                        out_d.ap()[
                            qt * SQT + qb * P : qt * SQT + (qb + 1) * P,
                            ec * SQT : (ec + 1) * SQT,
                        ],
                        osb[:],
                    )

        w_k, xk0 = load_w(wk_d, "wk", interleave_with=(xt_k, 0))
        if causal:
            # chunk 0 projections, then per q-tile: attend, project the next
            # chunk (fills attention + normalize PE bubbles), out-project
            proj_dt(w_k, xk0, kt_sb, 0)
            w_q, xq0 = load_w(wq_d, "wq", interleave_with=(xt_q, 0))
            qt_cur = qtp.tile([P, NJ, SQT], bf16, tag="qt", name="qt_t")
            proj_dt(w_q, xq0, qt_cur, 0)
            w_v, xv0 = load_w(wv_d, "wv", interleave_with=(xt_v, 0))
            proj_v_chunk(w_v, xv0, 0)
            nc.sync.dma_start(
                wo_sb[:], wo_d.ap().rearrange("(j p) e -> p j e", p=P)
            )
            deferred = []  # (qt, ot_tiles) with out-projection still to run
            for t in range(NQT):
                if t + 1 < NQT:
                    # issue next-chunk loads before attention so they are
                    # ahead in the Sync HWDGE FIFO
                    xk_n = load_xt(xt_k, t + 1)
                    xv_n = load_xt(xt_v, t + 1)
                    xq_n = load_xt(xt_q, t + 1)
                    fillers = ()
                else:
                    # last q-tile has no projection work left: spend the
                    # deferred out-projections here as PE filler
                    fillers = [
                        (lambda dq=dq, dot=dot: outproj_qt(dq, dot))
                        for dq, dot in deferred
                    ]
                pvc_all, inv_t = attn_scores_pv(t, qt_cur, fillers)
                if t + 1 < NQT:
                    proj_dt(w_k, xk_n, kt_sb, (t + 1) * SQT)
                    proj_v_chunk(w_v, xv_n, t + 1)
                    qt_next = qtp.tile([P, NJ, SQT], bf16, tag="qt", name="qt_t")
                    proj_dt(w_q, xq_n, qt_next, 0)
                ot_tiles = attn_norm(pvc_all, inv_t)
                if t + 1 < NQT:
                    deferred.append((t, ot_tiles))
                    qt_cur = qt_next
                else:
                    outproj_qt(t, ot_tiles)
        else:
            w_q, xq0 = load_w(wq_d, "wq", interleave_with=(xt_q, 0))
            w_v, xv0 = load_w(wv_d, "wv", interleave_with=(xt_v, 0))
            nc.sync.dma_start(
                wo_sb[:], wo_d.ap().rearrange("(j p) e -> p j e", p=P)
            )
            qts = []
            for sc in range(NQT):
                proj_dt(w_k, xk0 if sc == 0 else load_xt(xt_k, sc),
                        kt_sb, sc * SQT)
                proj_v_chunk(w_v, xv0 if sc == 0 else load_xt(xt_v, sc), sc)
                qt_t = qtp.tile([P, NJ, SQT], bf16, tag="qt", name="qt_t")
                proj_dt(w_q, xq0 if sc == 0 else load_xt(xt_q, sc), qt_t, 0)
                qts.append(qt_t)
            for t in range(NQT):
                pvc_all, inv_t = attn_scores_pv(t, qts[t])
                outproj_qt(t, attn_norm(pvc_all, inv_t))

    nc.compile()
    return nc


def _get_nc(causal: bool):
    if causal not in _CACHE:
        _CACHE[causal] = _build(causal)
    return _CACHE[causal]


def _numpy_ref(query, key, value, mask, wq, bq, wk, bk, wv, bv, wo, bo):
    """Exact fallback for inputs the device kernel doesn't specialize."""
    q = (query @ wq + bq).reshape(B, S, H, D).transpose(0, 2, 1, 3)
    k = (key @ wk + bk).reshape(B, S, H, D).transpose(0, 2, 1, 3)
    v = (value @ wv + bv).reshape(B, S, H, D).transpose(0, 2, 1, 3)
    out = np.empty((B, H, S, D), np.float32)
    for b in range(B):
        for h in range(H):
            s = q[b, h] @ k[b, h].T
            s = np.where(mask[b, 0], s, -np.inf) / np.sqrt(np.float32(D))
            s = s - s.max(axis=-1, keepdims=True)
            e = np.exp(s)
            out[b, h] = (e / e.sum(axis=-1, keepdims=True)) @ v[b, h]
    out = out.transpose(0, 2, 1, 3).reshape(B, S, E)
    return (out @ wo + bo).astype(np.float32)


def _make_in_maps(query, key, value, wq, wk, wv, wo):
    import ml_dtypes

    tri = np.ascontiguousarray(
        np.triu(np.ones((P, P), np.float32)).astype(ml_dtypes.bfloat16)
    )
    # selmat[r, j*128 + h2*64 + d] = 1 iff r == 64*(j%2) + 32*h2: broadcasts
    # den-tile row 64*(j%2)+32*h2 to rp rows [h2*64, h2*64+64) for pair j.
    selmat = np.zeros((P, NJ * P), np.float32)
    for j in range(NJ):
        for h2 in range(2):
            r = 64 * (j % 2) + 32 * h2
            selmat[r, j * P + h2 * D : j * P + (h2 + 1) * D] = 1.0
    bft = ml_dtypes.bfloat16
    in_maps = []
    for b in range(B):
        xq = np.ascontiguousarray(query[b].T.astype(bft))
        xk = np.ascontiguousarray(key[b].T.astype(bft))
        xv = np.ascontiguousarray(value[b].T.astype(bft))
        for half in (0, 1):
            cs = slice(half * EH, (half + 1) * EH)
            in_maps.append(
                {
                    "xt_q": xq,
                    "xt_k": xk,
                    "xt_v": xv,
                    "wq_h": np.ascontiguousarray(wq[:, cs].astype(bft)),
                    "wk_h": np.ascontiguousarray(wk[:, cs].astype(bft)),
                    "wv_h": np.ascontiguousarray(wv[:, cs].astype(bft)),
                    "wo_h": np.ascontiguousarray(
                        wo[cs, :].astype(ml_dtypes.bfloat16)
                    ),
                    "tri": tri,
                    "selmat": selmat,
                }
            )
    return in_maps


def benchmark(query, key, value, mask, wq, bq, wk, bk, wv, bv, wo, bo, iters=10):
    """Time repeated on-device executions with device-resident inputs.

    Returns (per_iter_seconds, outputs_like_kernel). Dispatch overhead through
    the axon tunnel is large (~10ms+), so this is an upper bound only.
    """
    import time
    import jax
    from jax.sharding import Mesh, PartitionSpec, NamedSharding
    from jax.experimental.shard_map import shard_map
    import concourse.mybir as mybir
    from concourse.bass2jax import (
        _bass_exec_p,
        install_neuronx_cc_hook,
        partition_id_tensor,
    )

    install_neuronx_cc_hook()
    query = np.asarray(query, np.float32)
    key = np.asarray(key, np.float32)
    value = np.asarray(value, np.float32)
    in_maps = _make_in_maps(query, key, value, wq, wk, wv, wo)
    nc = _get_nc(True)
    n_cores = 8

    partition_name = nc.partition_id_tensor.name if nc.partition_id_tensor else None
    in_names, out_names, out_avals, zero_outs = [], [], [], []
    for alloc in nc.m.functions[0].allocations:
        if not isinstance(alloc, mybir.MemoryLocationSet):
            continue
        name = alloc.memorylocations[0].name
        if alloc.kind == "ExternalInput":
            if name != partition_name:
                in_names.append(name)
        elif alloc.kind == "ExternalOutput":
            shape = tuple(alloc.tensor_shape)
            dtype = mybir.dt.np(alloc.dtype)
            out_names.append(name)
            out_avals.append(jax.core.ShapedArray(shape, dtype))
            zero_outs.append(np.zeros(shape, dtype))
    n_params = len(in_names)
    n_outs = len(out_avals)
    all_in_names = list(in_names) + out_names
    if partition_name is not None:
        all_in_names.append(partition_name)

    def _body(*args):
        operands = list(args)
        if partition_name is not None:
            operands.append(partition_id_tensor())
        return tuple(
            _bass_exec_p.bind(
                *operands,
                out_avals=tuple(out_avals),
                in_names=tuple(all_in_names),
                out_names=tuple(out_names),
                lowering_input_output_aliases=(),
                sim_require_finite=True,
                sim_require_nnan=True,
                nc=nc,
            )
        )

    devices = jax.devices()[:n_cores]
    mesh = Mesh(np.asarray(devices), ("core",))
    sharded = jax.jit(
        shard_map(
            _body,
            mesh=mesh,
            in_specs=(PartitionSpec("core"),) * (n_params + n_outs),
            out_specs=(PartitionSpec("core"),) * n_outs,
            check_rep=False,
        ),
        donate_argnums=tuple(range(n_params, n_params + n_outs)),
        keep_unused=True,
    )
    sh = NamedSharding(mesh, PartitionSpec("core"))
    concat_in = [
        jax.device_put(
            np.concatenate([np.asarray(in_maps[c][nm]) for c in range(n_cores)], 0), sh
        )
        for nm in in_names
    ]

    def fresh_zeros():
        return [
            jax.device_put(np.zeros((n_cores * z.shape[0], *z.shape[1:]), z.dtype), sh)
            for z in zero_outs
        ]

    outs = sharded(*concat_in, *fresh_zeros())
    jax.block_until_ready(outs)
    zsets = [fresh_zeros() for _ in range(iters)]
    for zs in zsets:
        jax.block_until_ready(zs)
    t0 = time.time()
    res = [sharded(*concat_in, *zs) for zs in zsets]
    jax.block_until_ready(res)
    dt = (time.time() - t0) / iters
    out_np = np.asarray(res[-1][out_names.index("out")]).reshape(n_cores, S, E)
    out = np.empty((B, S, E), np.float32)
    for b in range(B):
        out[b] = out_np[2 * b] + out_np[2 * b + 1]
    return dt, out


def kernel(query, key, value, mask, wq, bq, wk, bk, wv, bv, wo, bo):
    global LAST_RESULT
    query = np.asarray(query, np.float32)
    key = np.asarray(key, np.float32)
    value = np.asarray(value, np.float32)
    mask = np.asarray(mask)

    biases_zero = not (np.any(bq) or np.any(bk) or np.any(bv) or np.any(bo))
    m0 = mask[0, 0]
    tril = np.tril(np.ones((S, S), bool))
    if np.array_equal(m0, tril) and all(
        np.array_equal(mask[b, 0], m0) for b in range(1, B)
    ):
        causal = True
    elif mask.all():
        causal = False
    else:
        causal = None
    if query.shape != (B, S, E) or not biases_zero or causal is None:
        return _numpy_ref(
            query, key, value, mask, wq, bq, wk, bk, wv, bv, wo, bo
        )

    from concourse import bass_utils

    in_maps = _make_in_maps(query, key, value, wq, wk, wv, wo)
    nc = _get_nc(causal)
    res = bass_utils.run_bass_kernel_spmd(
        nc, in_maps, core_ids=list(range(8))
    )
    LAST_RESULT = res
    out = np.empty((B, S, E), np.float32)
    for b in range(B):
        out[b] = res.results[2 * b]["out"] + res.results[2 * b + 1]["out"]
    return out
